# revision 17
# baseline (speedup 1.0000x reference)
"""Trainium2 Bass kernel for nn_CRAMForCausalLM.

Sharding: 8-way data-parallel over tokens (each core owns 256 contiguous
tokens of one batch element plus an 8-token halo so the EMA retention scan
is computed locally — contributions older than 8 steps are damped by 0.5^8,
far below the error gate).  The LM head is pair-wise tensor parallel: cores
exchange final hiddens within pairs (0.5 MB AllGather) and each computes one
16000-row vocab half for the pair's 512 tokens.

LayerNorm handling: the mean subtraction is folded into the weights on the
host (column-mean-centered W annihilates the per-token mean of its input),
so the only runtime LN work is the variance epilogue: every GEMM reads the
raw pre-LN residual (bf16 cast) and its PSUM output is scaled by the
per-token rsqrt(var+eps) broadcast before the activation.  This keeps the
TensorEngine stream free of K=1 correction matmuls and lets GEMM phases
start without waiting for LN statistics.  Stats matmuls are emitted two
m-tiles behind their producers and phase boundaries are bridged by
deferring the last k-chunk of the first m-tiles of the next phase, so the
PE never idles (and never drops out of its top p-state).  Elementwise work
is spread over the Vector, Scalar and Pool (gpsimd) engines.
"""

import numpy as np

import concourse.bass as bass
import concourse.bacc as bacc
import concourse.tile as tile
import concourse.mybir as mybir
import concourse.bass_utils as bass_utils
import os as _os

LAST_EXEC_NS = None


def _maybe_install_trace_hook():
    import contextlib, ctypes, sys, types
    if "antenv.axon_hooks" in sys.modules:
        return
    lib = ctypes.CDLL("/opt/axon/libaxon_pjrt.so")
    if not hasattr(lib, "axon_start_nrt_profile"):
        return
    lib.axon_start_nrt_profile.argtypes = [ctypes.POINTER(ctypes.c_int64), ctypes.c_size_t]
    lib.axon_start_nrt_profile.restype = ctypes.c_int64
    lib.axon_stop_nrt_profile.argtypes = [ctypes.c_char_p]
    lib.axon_stop_nrt_profile.restype = ctypes.c_int64

    @contextlib.contextmanager
    def _hook(output_dir, device_ids):
        import jax
        jax.devices()
        if device_ids:
            ids = (ctypes.c_int64 * len(device_ids))(*device_ids)
            rc = lib.axon_start_nrt_profile(ids, len(device_ids))
        else:
            rc = lib.axon_start_nrt_profile(None, 0)
        if rc != 0:
            raise RuntimeError(f"axon_start_nrt_profile rc={rc}")
        try:
            yield
        finally:
            lib.axon_stop_nrt_profile(str(output_dir).encode())

    mod = types.ModuleType("antenv.axon_hooks")
    mod.get_axon_ntff_profile_hook = lambda: _hook
    mod.set_axon_ntff_profile_hook = lambda h: None
    sys.modules["antenv.axon_hooks"] = mod

AF = mybir.ActivationFunctionType
OP = mybir.AluOpType

B, S, H, F, L, V = 2, 1024, 1024, 4096, 8, 32000
EPS = 1e-5
NCORES = 8
HALO = 8
TM = 256            # main tokens per core
T = TM + HALO       # 264 tokens processed per core
KH = H // 128       # 8 k-chunks over H
MH = H // 128       # 8 m-tiles over H
MF = F // 128       # 32 m-tiles over F
VM = V // 128       # 250 vocab m-tiles

f32 = mybir.dt.float32
f32r = mybir.dt.float32r
bf16 = mybir.dt.bfloat16
f16 = mybir.dt.float16

_compiled = {}


def _swz(w, kp=128, mf=128):
    """[K, M] -> [mt, kp, kc*mf] so lhsT tile (mt, kc) = sbuf[:, kc*mf:(kc+1)*mf]."""
    K, M = w.shape
    kc, mt = K // kp, M // mf
    return np.ascontiguousarray(
        w.reshape(kc, kp, mt, mf).transpose(2, 1, 0, 3).reshape(mt, kp, kc * mf)
    )


def _cols(v, mt, width=128):
    """[M] -> [width, mt] so column j is v[j*width:(j+1)*width]."""
    return np.ascontiguousarray(v.reshape(mt, width).T)


def _build(ln_scaled):
    nc = bacc.Bacc("TRN2", target_bir_lowering=False, debug=False,
                   num_devices=NCORES)

    # ---- DRAM I/O ----
    xemb_d = nc.dram_tensor("xemb", [KH, 128, T], f32, kind="ExternalInput")
    retw_d = nc.dram_tensor("retw", [L, MH, 128, KH * 128], bf16, kind="ExternalInput")
    retb_d = nc.dram_tensor("retb", [L, 128, MH], f32, kind="ExternalInput")
    w1_d = nc.dram_tensor("w1", [L, MF, 128, KH * 128], bf16, kind="ExternalInput")
    b1_d = nc.dram_tensor("b1", [L, 128, MF], f32, kind="ExternalInput")
    w2_d = nc.dram_tensor("w2", [L, MH, 128, MF * 128], bf16, kind="ExternalInput")
    b2_d = nc.dram_tensor("b2", [L, 128, MH], f32, kind="ExternalInput")
    lmw_d = nc.dram_tensor("lmw", [VM // 2, 128, KH * 128], bf16, kind="ExternalInput")
    mask_d = nc.dram_tensor("mask", [128, 1], f32, kind="ExternalInput")
    if ln_scaled:
        lns_d = nc.dram_tensor("lns", [2 * L + 2, 2, 128, MH], f32, kind="ExternalInput")
    out_d = nc.dram_tensor("logits", [VM // 2, 128, 2 * TM], f16,
                           kind="ExternalOutput")
    PAIRS = [[2 * i, 2 * i + 1] for i in range(NCORES // 2)]

    with tile.TileContext(nc) as tc:
        with tc.tile_pool(name="per", bufs=1) as per, \
             tc.tile_pool(name="gpool", bufs=1) as gpool, \
             tc.tile_pool(name="lnst", bufs=2) as lnst:
            # persistent activation tiles
            xpre = [per.tile([128, T], f32, tag=f"xp{k}", name=f"xp{k}") for k in range(KH)]
            xt = [per.tile([128, T], f32, tag=f"xt{k}", name=f"xt{k}") for k in range(KH)]
            y1 = [per.tile([128, T], f32, tag=f"y1{k}", name=f"y1{k}") for k in range(KH)]
            hres = [per.tile([128, T], f32, tag=f"h{k}", name=f"h{k}") for k in range(KH)]
            yb1 = [per.tile([128, T], bf16, tag=f"yb1{k}", name=f"yb1{k}") for k in range(KH)]
            yb2 = [per.tile([128, T], bf16, tag=f"yb2{k}", name=f"yb2{k}") for k in range(KH)]
            g = [gpool.tile([128, T], bf16, tag=f"g{k}", name=f"g{k}") for k in range(MF)]
            half_f = per.tile([128, T], f32)
            nc.gpsimd.memset(half_f[:], 0.5)
            half = per.tile([128, T], bf16)
            nc.vector.tensor_copy(half[:], half_f[:])
            ones_f = per.tile([128, 1], f32)
            nc.gpsimd.memset(ones_f[:], 1.0)
            ones = per.tile([128, 1], bf16)
            nc.vector.tensor_copy(ones[:], ones_f[:])
            onesr_f = per.tile([1, 128], f32)
            nc.gpsimd.memset(onesr_f[:], 1.0)
            onesr = per.tile([1, 128], f32r)
            nc.vector.tensor_copy(onesr[:], onesr_f[:])
            mask = per.tile([128, 1], f32)
            nc.sync.dma_start(mask[:], mask_d.ap())
            m01f = per.tile([128, HALO], f32)
            nc.gpsimd.memset(m01f[:], 1.0)
            nc.vector.tensor_scalar_mul(m01f[:], m01f[:], mask[:, :1])
            mask01 = per.tile([128, HALO], bf16)
            nc.vector.tensor_copy(mask01[:], m01f[:])
            epsc = per.tile([128, 1], f32)
            nc.gpsimd.memset(epsc[:], EPS)
            if ln_scaled:
                lnt = per.tile([128, (2 * L + 2) * 2 * MH], f32)
                nc.sync.dma_start(
                    lnt[:],
                    lns_d.ap().rearrange("a b p m -> p (a b m)"))
            else:
                lnt = None

            def ln_cols(slot):
                if lnt is None:
                    return None, None
                off = slot * 2 * MH
                return lnt[:, off:off + MH], lnt[:, off + MH:off + 2 * MH]

            # ================= Embedding =================
            with tc.tile_pool(name="dramw", bufs=1, space="DRAM") as dramw:
                # tiny warm-up AllGather to absorb collective setup cost
                win = dramw.tile([128, 4], f32)
                nc.sync.dma_start(win[:], half_f[:, :4])
                wout = dramw.tile([2, 128, 4], f32)
                nc.gpsimd.collective_compute(
                    "AllGather", OP.bypass, replica_groups=PAIRS,
                    ins=[win.opt()], outs=[wout.opt()])
                for k in range(KH):
                    nc.sync.dma_start(xpre[k][:], xemb_d.ap()[k])

            # ================= Layers =================
            with tc.tile_pool(name="wret", bufs=6) as wret, \
                 tc.tile_pool(name="w1p", bufs=6) as w1p, \
                 tc.tile_pool(name="w2p", bufs=4) as w2p, \
                 tc.tile_pool(name="bias", bufs=2) as biasp, \
                 tc.tile_pool(name="tmp", bufs=4) as tmp, \
                 tc.tile_pool(name="sqp", bufs=3) as sqp, \
                 tc.tile_pool(name="psmm", bufs=4, space="PSUM") as psmm, \
                 tc.tile_pool(name="psst", bufs=2, space="PSUM") as ps_stat, \
                 tc.tile_pool(name="psbc", bufs=2, space="PSUM") as ps_bc:

                def stats_open():
                    # p_sy on partition 0, p_sq on partition 32: one PSUM bank
                    st = ps_stat.tile([33, T], f32, tag="pst", name="p_st")
                    return st

                def cast_sq(mt, src, yb):
                    """bf16 cast + square, both on the Scalar engine (Copy and
                    Square live in every activation table set)."""
                    nc.scalar.copy(yb[mt][:], src[mt][:])
                    sq = sqp.tile([128, T], bf16, tag="sq", name=f"sq{mt}")
                    nc.scalar.square(sq[:], yb[mt][:])
                    return sq

                def stats_mm(st_ps, mt, yb, sq, n):
                    nc.tensor.matmul(st_ps[0:1, :], ones[:], yb[mt][:],
                                     start=(mt == 0), stop=(mt == n - 1))
                    nc.tensor.matmul(st_ps[32:33, :], ones[:], sq[:],
                                     start=(mt == 0), stop=(mt == n - 1))

                def ln_finish(st_ps):
                    """Produce rb_sb ([128,T] rsqrt(var+eps)) and nmb_sb
                    ([128,T] negative mean) in SBUF."""
                    nm = lnst.tile([1, T], f32r, tag="nm", name="nm")
                    nc.vector.tensor_scalar_mul(nm[:], st_ps[0:1, :], -1.0 / H)
                    m2 = lnst.tile([1, T], f32, tag="m2", name="m2")
                    nc.vector.tensor_tensor(m2[:], nm[:].bitcast(f32),
                                            nm[:].bitcast(f32), OP.mult)
                    var = lnst.tile([1, T], f32r, tag="var", name="var")
                    nc.vector.scalar_tensor_tensor(var[:], st_ps[32:33, :],
                                                   1.0 / H, m2[:],
                                                   OP.mult, OP.subtract)
                    p_vb = ps_bc.tile([128, T], f32, tag="bc", name="p_vb")
                    nc.tensor.matmul(p_vb[:], onesr[:], var[:],
                                     start=True, stop=True)
                    p_nmb = ps_bc.tile([128, T], f32, tag="bc", name="p_nmb")
                    nc.tensor.matmul(p_nmb[:], onesr[:], nm[:],
                                     start=True, stop=True)
                    rb_sb = lnst.tile([128, T], f32, tag="rb", name="rb_sb")
                    nc.scalar.activation(rb_sb[:], p_vb[:],
                                         AF.Abs_reciprocal_sqrt, bias=epsc[:])
                    nmb_sb = lnst.tile([128, T], f32, tag="nmb", name="nmb_sb")
                    nc.scalar.copy(nmb_sb[:], p_nmb[:])
                    return {"rb": rb_sb, "nmb": nmb_sb}

                def apply_ln(mt, src, st, dst, slot):
                    """dst[mt] = (src[mt] + nmb)*rb (*s + b) on the Pool engine."""
                    z = tmp.tile([128, T], f32, tag="z", name="z")
                    nc.gpsimd.tensor_tensor(z[:], src[mt][:], st["nmb"][:], OP.add)
                    scol, bcol = ln_cols(slot)
                    if scol is None:
                        nc.gpsimd.tensor_tensor(dst[mt][:], z[:], st["rb"][:],
                                                OP.mult)
                    else:
                        z2 = tmp.tile([128, T], f32, tag="z2", name="z2")
                        nc.gpsimd.tensor_tensor(z2[:], z[:], st["rb"][:], OP.mult)
                        nc.gpsimd.tensor_scalar(dst[mt][:], z2[:],
                                                scol[:, mt:mt + 1],
                                                bcol[:, mt:mt + 1],
                                                OP.mult, OP.add)

                def epi(ps, st, out, func, bias):
                    """out = func(ps*rb + bias)."""
                    fin = tmp.tile([128, T], f32, tag="epf", name="epf")
                    nc.vector.tensor_tensor(fin[:], ps[:], st["rb"][:], OP.mult)
                    nc.scalar.activation(out, fin[:], func, bias=bias)

                # ---- embedding LN stats (emb acts as layer -1's LN2) ----
                emb_ps = stats_open()
                for k in range(KH):
                    sq = cast_sq(k, xpre, yb2)
                    stats_mm(emb_ps, k, yb2, sq, KH)
                st2 = ln_finish(emb_ps)
                # (st2_ps_pend, sq7) deferred from the previous layer's ffn2 so
                # the next ret head can be emitted between its PE instructions
                pend = None

                for l in range(L):
                    retb = biasp.tile([128, MH], f32, tag="retb")
                    nc.sync.dma_start(retb[:], retb_d.ap()[l])
                    b1 = biasp.tile([128, MF], f32, tag="b1")
                    nc.sync.dma_start(b1[:], b1_d.ap()[l])
                    b2 = biasp.tile([128, MH], f32, tag="b2")
                    nc.sync.dma_start(b2[:], b2_d.ap()[l])

                    # ---------- retention ----------
                    st1_ps = stats_open()
                    sq_pend = {}      # mt -> sq tile (stats emitted at lag 2)
                    for mt in range(MH):
                        wt = wret.tile([128, KH * 128], bf16, tag="wret")
                        nc.sync.dma_start(wt[:], retw_d.ap()[l, mt])
                        ps = psmm.tile([128, T], f32, tag="mm")
                        if mt == 0 and pend is not None:
                            # head: kc0-6 fill the PE while yb2[7]'s chain and
                            # the previous LN2 finish complete
                            for kc in range(7):
                                nc.tensor.matmul(
                                    ps[:], wt[:, kc * 128:(kc + 1) * 128],
                                    yb2[kc][:], start=(kc == 0), stop=False)
                            dmy = ps_bc.tile([1, T], f32, tag="bc", name="dmy")
                            for _ in range(10):
                                nc.tensor.matmul(dmy[:], ones[:], yb2[0][:],
                                                 start=True, stop=True)
                            p_st2_ps, p_sq7 = pend
                            pend = None
                            stats_mm(p_st2_ps, MH - 1, yb2, p_sq7, MH)
                            st2 = ln_finish(p_st2_ps)
                            nc.tensor.matmul(
                                ps[:], wt[:, 7 * 128:8 * 128], yb2[7][:],
                                start=False, stop=True)
                        else:
                            for kc in range(KH):
                                nc.tensor.matmul(
                                    ps[:], wt[:, kc * 128:(kc + 1) * 128],
                                    yb2[kc][:], start=(kc == 0),
                                    stop=(kc == KH - 1))
                        s = tmp.tile([128, T], bf16, tag="sig", name="sig")
                        epi(ps[:], st2, s[:], AF.Sigmoid, retb[:, mt:mt + 1])
                        nc.vector.tensor_tensor(
                            s[:, :HALO], s[:, :HALO], mask01[:], OP.mult)
                        # xt[mt] = LN2(xpre[mt]) just in time for the residual
                        apply_ln(mt, xpre, st2, xt,
                                 (2 * l) if ln_scaled else 0)
                        stt = tmp.tile([128, T], bf16, tag="scan", name="scan")
                        nc.vector.tensor_tensor_scan(
                            stt[:], half[:], s[:], 0.0, OP.mult, OP.add)
                        nc.vector.scalar_tensor_tensor(
                            y1[mt][:], stt[:], 0.5, xt[mt][:], OP.mult, OP.add)
                        sq_pend[mt] = cast_sq(mt, y1, yb1)

                    # ffn1 head: 4 m-tiles kc-major, interleaved with the LN1
                    # stats so the PE consumes the retention chain's outputs
                    # in production order instead of stalling on them.  Dummy
                    # matmuls (always-ready inputs, dead PSUM target) pad the
                    # PE stream so it never idles waiting for the chain —
                    # idling drops the PE to a lower p-state and the first
                    # ~3us after each gap would run at half clock.
                    NG = 4
                    DUMS = [0, 0, 8, 12, 14, 14, 14, 14]
                    fwt = []
                    fps = []
                    for mt in range(NG):
                        wt = w1p.tile([128, KH * 128], bf16, tag="w1")
                        nc.sync.dma_start(wt[:], w1_d.ap()[l, mt])
                        fwt.append(wt)
                        ps = psmm.tile([128, T], f32, tag="mm")
                        fps.append(ps)
                    for kc in range(KH):
                        for mt in range(NG):
                            nc.tensor.matmul(
                                fps[mt][:], fwt[mt][:, kc * 128:(kc + 1) * 128],
                                yb1[kc][:], start=(kc == 0), stop=(kc == KH - 1))
                        if DUMS[kc]:
                            dmy = ps_bc.tile([1, T], f32, tag="bc", name="dmy")
                            for _ in range(DUMS[kc]):
                                nc.tensor.matmul(dmy[:], ones[:], yb2[0][:],
                                                 start=True, stop=True)
                        stats_mm(st1_ps, kc, yb1, sq_pend.pop(kc), MH)
                    st1 = ln_finish(st1_ps)

                    # ---------- FFN1 ----------
                    for mt in range(MF):
                        if mt < NG:
                            ps = fps[mt]
                        else:
                            wt = w1p.tile([128, KH * 128], bf16, tag="w1")
                            nc.sync.dma_start(wt[:], w1_d.ap()[l, mt])
                            ps = psmm.tile([128, T], f32, tag="mm")
                            for kc in range(KH):
                                nc.tensor.matmul(
                                    ps[:], wt[:, kc * 128:(kc + 1) * 128], yb1[kc][:],
                                    start=(kc == 0), stop=(kc == KH - 1))
                        epi(ps[:], st1, g[mt][:], AF.Gelu_apprx_tanh,
                            b1[:, mt:mt + 1])
                        if mt < MH:
                            # h[mt] = LN1(y1[mt]) for the ffn2 residual
                            apply_ln(mt, y1, st1, hres,
                                     (2 * l + 1) if ln_scaled else 0)

                    # ---------- FFN2 ----------
                    # last layer (identity path): only the bf16 casts are
                    # needed — the final LN scale is recomputed from the
                    # gathered casts on the LM side, so skip stats entirely
                    last_id = (l == L - 1) and not ln_scaled
                    st2_ps = None if last_id else stats_open()
                    sq_pend = {}
                    for mt in range(MH):
                        wt = w2p.tile([128, MF * 128], bf16, tag="w2")
                        nc.sync.dma_start(wt[:], w2_d.ap()[l, mt])
                        ps = psmm.tile([128, T], f32, tag="mm")
                        for kc in range(MF):
                            nc.tensor.matmul(
                                ps[:], wt[:, kc * 128:(kc + 1) * 128], g[kc][:],
                                start=(kc == 0), stop=(kc == MF - 1))
                        # xpre' = (ffn + b2) + h
                        nc.vector.scalar_tensor_tensor(
                            xpre[mt][:], ps[:], b2[:, mt:mt + 1],
                            hres[mt][:], OP.add, OP.add)
                        if last_id:
                            nc.scalar.copy(yb2[mt][:], xpre[mt][:])
                            continue
                        sq_pend[mt] = cast_sq(mt, xpre, yb2)
                        if mt >= 2:
                            stats_mm(st2_ps, mt - 2, yb2, sq_pend.pop(mt - 2), MH)

                    # stats(6) now; stats(7) + ln2 finish are deferred into the
                    # next layer's ret head so the PE keeps streaming
                    if not last_id:
                        stats_mm(st2_ps, MH - 2, yb2, sq_pend.pop(MH - 2), MH)
                        if l < L - 1:
                            pend = (st2_ps, sq_pend.pop(MH - 1))
                        else:
                            stats_mm(st2_ps, MH - 1, yb2, sq_pend.pop(MH - 1), MH)
                            st2 = ln_finish(st2_ps)

                # ---- final LN (identity path: LM reads xpre*rb directly) ----
                if ln_scaled:
                    # materialize the scaled LN2 output, then run the final LN
                    for k in range(KH):
                        apply_ln(k, xpre, st2, xt, 2 * L)
                    stf_ps = stats_open()
                    sq_pend = {}
                    for k in range(KH):
                        sq = cast_sq(k, xt, yb1)
                        stats_mm(stf_ps, k, yb1, sq, KH)
                    stf = ln_finish(stf_ps)
                    for k in range(KH):
                        apply_ln(k, xt, stf, hres, 2 * L + 1)
                    xlm = yb2
                    for k in range(KH):
                        nc.scalar.copy(xlm[k][:], hres[k][:])
                else:
                    # raw bf16 casts are gathered; the final LN rsqrt is
                    # recomputed on the receiving side for all pair tokens
                    xlm = yb2

            # ===== LM head: 2-way vocab shard x pair token gather =====
            with tc.tile_pool(name="dram", bufs=1, space="DRAM") as dramp, \
                 tc.tile_pool(name="lmx", bufs=1) as lmx, \
                 tc.tile_pool(name="lmsq", bufs=2) as lmsq, \
                 tc.tile_pool(name="lmw", bufs=10) as lmwp, \
                 tc.tile_pool(name="lmo", bufs=4) as lmo, \
                 tc.tile_pool(name="pslm", bufs=6, space="PSUM") as pslm, \
                 tc.tile_pool(name="pslst", bufs=1, space="PSUM") as ps_lst:
                bnc = dramp.tile([H, TM], bf16)
                for k in range(KH):
                    nc.sync.dma_start(bnc[k * 128:(k + 1) * 128, :],
                                      xlm[k][:, HALO:T])
                xg = dramp.tile([2, H, TM], bf16)
                nc.gpsimd.collective_compute(
                    "AllGather", OP.bypass, replica_groups=PAIRS,
                    ins=[bnc.opt()], outs=[xg.opt()])
                rhs = []
                for k in range(KH):
                    t_ = lmx.tile([128, 2 * TM], bf16, tag=f"rhs{k}",
                                  name=f"rhs{k}")
                    for r in range(2):
                        nc.sync.dma_start(
                            t_[:, r * TM:(r + 1) * TM],
                            xg[r, k * 128:(k + 1) * 128, :])
                    rhs.append(t_)
                if not ln_scaled:
                    # recompute the final-LN rsqrt for the pair's 512 tokens
                    lm_st = ps_lst.tile([33, 2 * TM], f32, tag="lmst",
                                        name="lm_st")
                    for k in range(KH):
                        sq = lmsq.tile([128, 2 * TM], bf16, tag="lsq",
                                       name="lsq")
                        nc.scalar.square(sq[:], rhs[k][:])
                        nc.tensor.matmul(lm_st[0:1, :], ones[:], rhs[k][:],
                                         start=(k == 0), stop=(k == KH - 1))
                        nc.tensor.matmul(lm_st[32:33, :], ones[:], sq[:],
                                         start=(k == 0), stop=(k == KH - 1))
                    nm = lmsq.tile([1, 2 * TM], f32r, tag="lnm", name="lnm")
                    nc.vector.tensor_scalar_mul(nm[:], lm_st[0:1, :], -1.0 / H)
                    m2 = lmsq.tile([1, 2 * TM], f32, tag="lm2", name="lm2")
                    nc.vector.tensor_tensor(m2[:], nm[:].bitcast(f32),
                                            nm[:].bitcast(f32), OP.mult)
                    var = lmsq.tile([1, 2 * TM], f32r, tag="lvar", name="lvar")
                    nc.vector.scalar_tensor_tensor(var[:], lm_st[32:33, :],
                                                   1.0 / H, m2[:],
                                                   OP.mult, OP.subtract)
                    p_vb = ps_lst.tile([128, 2 * TM], f32, tag="lbc",
                                       name="p_vb")
                    nc.tensor.matmul(p_vb[:], onesr[:], var[:],
                                     start=True, stop=True)
                    rb = lmsq.tile([128, 2 * TM], f32, tag="lrb", name="lrb")
                    nc.scalar.activation(rb[:], p_vb[:],
                                         AF.Abs_reciprocal_sqrt, bias=epsc[:])
                    for k in range(KH):
                        nrm = lmx.tile([128, 2 * TM], bf16, tag=f"nrm{k}",
                                       name=f"nrm{k}")
                        nc.vector.tensor_tensor(nrm[:], rhs[k][:], rb[:],
                                                OP.mult)
                        rhs[k] = nrm
                for mt in range(VM // 2):
                    wt = lmwp.tile([128, KH * 128], bf16, tag="lmw")
                    nc.sync.dma_start(wt[:], lmw_d.ap()[mt])
                    ps = pslm.tile([128, 2 * TM], f32, tag="lm")
                    for kc in range(KH):
                        nc.tensor.matmul(
                            ps[:], wt[:, kc * 128:(kc + 1) * 128], rhs[kc][:],
                            start=(kc == 0), stop=(kc == KH - 1))
                    ob = lmo.tile([128, 2 * TM], f16, tag="ob")
                    nc.any.tensor_copy(ob[:], ps[:])
                    nc.sync.dma_start(out_d.ap()[mt], ob[:])

    nc.compile()
    return nc


def _prep_inputs(inputs, ln_scaled):
    import ml_dtypes
    ids = np.asarray(inputs["input_ids"], np.int32)          # [B, S]
    retw_raw = [np.asarray(inputs["ret_W"][l], np.float32) for l in range(L)]
    w1_raw = [np.asarray(inputs["ffn_W1"][l], np.float32) for l in range(L)]
    retb_raw = [np.asarray(inputs["ret_b"][l], np.float32) for l in range(L)]
    b1_raw = [np.asarray(inputs["ffn_b1"][l], np.float32) for l in range(L)]
    if ln_scaled:
        # fold LN scale/bias of the LN feeding each fused GEMM into W / bias
        for l in range(L):
            s_in = (np.asarray(inputs["emb_ln_s"], np.float32) if l == 0
                    else np.asarray(inputs["ln2_s"][l - 1], np.float32))
            b_in = (np.asarray(inputs["emb_ln_b"], np.float32) if l == 0
                    else np.asarray(inputs["ln2_b"][l - 1], np.float32))
            retb_raw[l] = retb_raw[l] + b_in @ retw_raw[l]
            retw_raw[l] = retw_raw[l] * s_in[:, None]
            s1 = np.asarray(inputs["ln1_s"][l], np.float32)
            b1_ = np.asarray(inputs["ln1_b"][l], np.float32)
            b1_raw[l] = b1_raw[l] + b1_ @ w1_raw[l]
            w1_raw[l] = w1_raw[l] * s1[:, None]
    # column-mean-center: W~ = W - 1 (x) colmean(W) makes the GEMM
    # invariant to the per-token mean of its input
    retw_raw = [w - w.mean(0, keepdims=True) for w in retw_raw]
    w1_raw = [w - w.mean(0, keepdims=True) for w in w1_raw]
    retw = np.stack([_swz(w) for w in retw_raw]).astype(ml_dtypes.bfloat16)
    w1 = np.stack([_swz(w) for w in w1_raw]).astype(ml_dtypes.bfloat16)
    w2 = np.stack([_swz(np.asarray(inputs["ffn_W2"][l], np.float32))
                   for l in range(L)]).astype(ml_dtypes.bfloat16)
    retb = np.stack([_cols(v, MH) for v in retb_raw])
    b1 = np.stack([_cols(v, MF) for v in b1_raw])
    b2 = np.stack([_cols(np.asarray(inputs["ffn_b2"][l], np.float32), MH)
                   for l in range(L)])
    lmw_full = np.asarray(inputs["lm_W"], np.float32)         # [H, V]
    if not ln_scaled:
        lmw_full = lmw_full - lmw_full.mean(0, keepdims=True)
    pos_emb = np.asarray(inputs["pos_emb"], np.float32)       # [S, H]
    wemb = np.asarray(inputs["word_emb"], np.float32)

    common = {
        "retw": retw, "retb": retb,
        "w1": w1, "b1": b1, "w2": w2, "b2": b2,
    }
    lmw_halves = [
        _swz(np.ascontiguousarray(lmw_full[:, h * (V // 2):(h + 1) * (V // 2)])
             ).astype(ml_dtypes.bfloat16)
        for h in range(2)
    ]
    if ln_scaled:
        slots = [( np.asarray(inputs["emb_ln_s"], np.float32),
                   np.asarray(inputs["emb_ln_b"], np.float32))]
        for l in range(L):
            slots.append((np.asarray(inputs["ln1_s"][l], np.float32),
                          np.asarray(inputs["ln1_b"][l], np.float32)))
            slots.append((np.asarray(inputs["ln2_s"][l], np.float32),
                          np.asarray(inputs["ln2_b"][l], np.float32)))
        slots.append((np.asarray(inputs["fin_ln_s"], np.float32),
                      np.asarray(inputs["fin_ln_b"], np.float32)))
        lns = np.stack([np.stack([_cols(s, MH), _cols(b, MH)]) for s, b in slots])
        common["lns"] = lns

    in_maps = []
    for c in range(NCORES):
        b = c // (NCORES // B)
        s0 = TM * (c % (NCORES // B))
        if s0 == 0:
            hids = ids[b, 0:HALO]
            hpos = np.arange(HALO)
        else:
            hids = ids[b, s0 - HALO:s0]
            hpos = np.arange(s0 - HALO, s0)
        cids = np.concatenate([hids, ids[b, s0:s0 + TM]]).astype(np.int64)
        cpos = np.concatenate([hpos, np.arange(s0, s0 + TM)])
        xemb = (wemb[cids] + pos_emb[cpos]).astype(np.float32)   # [T, H]
        m = dict(common)
        m["mask"] = np.full((128, 1), 0.0 if s0 == 0 else 1.0, np.float32)
        m["xemb"] = np.ascontiguousarray(xemb.T.reshape(KH, 128, T))
        m["lmw"] = lmw_halves[c % 2]
        in_maps.append(m)
    return in_maps


def kernel(**inputs):
    trivial = all(
        np.allclose(np.asarray(inputs[k]), 1.0) for k in
        ("emb_ln_s", "ln1_s", "ln2_s", "fin_ln_s")
    ) and all(
        np.allclose(np.asarray(inputs[k]), 0.0) for k in
        ("emb_ln_b", "ln1_b", "ln2_b", "fin_ln_b")
    )
    ln_scaled = not trivial

    if ln_scaled not in _compiled:
        _compiled[ln_scaled] = _build(ln_scaled)
    nc = _compiled[ln_scaled]

    in_maps = _prep_inputs(inputs, ln_scaled)
    trace = bool(_os.environ.get("KERNEL_TRACE"))
    if trace:
        _maybe_install_trace_hook()
    res = bass_utils.run_bass_kernel_spmd(
        nc, in_maps, core_ids=list(range(NCORES)), trace=trace)
    global LAST_EXEC_NS
    LAST_EXEC_NS = res.exec_time_ns

    logits = np.empty((B, S, V), np.float32)
    for c in range(NCORES):
        b = c // (NCORES // B)
        s0 = 2 * TM * ((c // 2) % 2)
        vh = c % 2
        lg = np.asarray(res.results[c]["logits"])  # [125, 128, 512] f16
        logits[b, s0:s0 + 2 * TM, vh * (V // 2):(vh + 1) * (V // 2)] = \
            lg.reshape(V // 2, 2 * TM).astype(np.float32).T
    return logits


# revision 20
# speedup vs baseline: 1.0391x; 1.0391x over previous
"""Trainium2 Bass kernel for nn_CRAMForCausalLM.

Sharding: 8-way data-parallel over tokens (each core owns 256 contiguous
tokens of one batch element plus an 8-token halo so the EMA retention scan
is computed locally — contributions older than 8 steps are damped by 0.5^8,
far below the error gate).  The LM head is pair-wise tensor parallel: cores
exchange final hiddens within pairs (0.5 MB AllGather) and each computes one
16000-row vocab half for the pair's 512 tokens.

LayerNorm handling: the mean subtraction is folded into the weights on the
host (column-mean-centered W annihilates the per-token mean of its input),
so the only runtime LN work is the variance epilogue: every GEMM reads the
raw pre-LN residual (bf16 cast) and its PSUM output is scaled by the
per-token rsqrt(var+eps) broadcast before the activation.  This keeps the
TensorEngine stream free of K=1 correction matmuls and lets GEMM phases
start without waiting for LN statistics.  Stats matmuls are emitted two
m-tiles behind their producers and phase boundaries are bridged by
deferring the last k-chunk of the first m-tiles of the next phase, so the
PE never idles (and never drops out of its top p-state).  Elementwise work
is spread over the Vector, Scalar and Pool (gpsimd) engines.
"""

import numpy as np

import concourse.bass as bass
import concourse.bacc as bacc
import concourse.tile as tile
import concourse.mybir as mybir
import concourse.bass_utils as bass_utils
import os as _os

LAST_EXEC_NS = None


def _maybe_install_trace_hook():
    import contextlib, ctypes, sys, types
    if "antenv.axon_hooks" in sys.modules:
        return
    lib = ctypes.CDLL("/opt/axon/libaxon_pjrt.so")
    if not hasattr(lib, "axon_start_nrt_profile"):
        return
    lib.axon_start_nrt_profile.argtypes = [ctypes.POINTER(ctypes.c_int64), ctypes.c_size_t]
    lib.axon_start_nrt_profile.restype = ctypes.c_int64
    lib.axon_stop_nrt_profile.argtypes = [ctypes.c_char_p]
    lib.axon_stop_nrt_profile.restype = ctypes.c_int64

    @contextlib.contextmanager
    def _hook(output_dir, device_ids):
        import jax
        jax.devices()
        if device_ids:
            ids = (ctypes.c_int64 * len(device_ids))(*device_ids)
            rc = lib.axon_start_nrt_profile(ids, len(device_ids))
        else:
            rc = lib.axon_start_nrt_profile(None, 0)
        if rc != 0:
            raise RuntimeError(f"axon_start_nrt_profile rc={rc}")
        try:
            yield
        finally:
            lib.axon_stop_nrt_profile(str(output_dir).encode())

    mod = types.ModuleType("antenv.axon_hooks")
    mod.get_axon_ntff_profile_hook = lambda: _hook
    mod.set_axon_ntff_profile_hook = lambda h: None
    sys.modules["antenv.axon_hooks"] = mod

AF = mybir.ActivationFunctionType
OP = mybir.AluOpType

B, S, H, F, L, V = 2, 1024, 1024, 4096, 8, 32000
EPS = 1e-5
NCORES = 8
HALO = 8
TM = 256            # main tokens per core
T = TM + HALO       # 264 tokens processed per core
KH = H // 128       # 8 k-chunks over H
MH = H // 128       # 8 m-tiles over H
MF = F // 128       # 32 m-tiles over F
VM = V // 128       # 250 vocab m-tiles

f32 = mybir.dt.float32
f32r = mybir.dt.float32r
bf16 = mybir.dt.bfloat16
f16 = mybir.dt.float16

_compiled = {}


def _swz(w, kp=128, mf=128):
    """[K, M] -> [mt, kp, kc*mf] so lhsT tile (mt, kc) = sbuf[:, kc*mf:(kc+1)*mf]."""
    K, M = w.shape
    kc, mt = K // kp, M // mf
    return np.ascontiguousarray(
        w.reshape(kc, kp, mt, mf).transpose(2, 1, 0, 3).reshape(mt, kp, kc * mf)
    )


def _cols(v, mt, width=128):
    """[M] -> [width, mt] so column j is v[j*width:(j+1)*width]."""
    return np.ascontiguousarray(v.reshape(mt, width).T)


def _build(ln_scaled):
    nc = bacc.Bacc("TRN2", target_bir_lowering=False, debug=False,
                   num_devices=NCORES)

    # ---- DRAM I/O ----
    xemb_d = nc.dram_tensor("xemb", [KH, 128, T], f32, kind="ExternalInput")
    retw_d = nc.dram_tensor("retw", [L, MH, 128, KH * 128], bf16, kind="ExternalInput")
    retb_d = nc.dram_tensor("retb", [L, 128, MH], f32, kind="ExternalInput")
    w1_d = nc.dram_tensor("w1", [L, MF, 128, KH * 128], bf16, kind="ExternalInput")
    b1_d = nc.dram_tensor("b1", [L, 128, MF], f32, kind="ExternalInput")
    w2_d = nc.dram_tensor("w2", [L, MH, 128, MF * 128], bf16, kind="ExternalInput")
    b2_d = nc.dram_tensor("b2", [L, 128, MH], f32, kind="ExternalInput")
    lmw_d = nc.dram_tensor("lmw", [VM // 2, 128, KH * 128], bf16, kind="ExternalInput")
    mask_d = nc.dram_tensor("mask", [128, 1], f32, kind="ExternalInput")
    if ln_scaled:
        lns_d = nc.dram_tensor("lns", [2 * L + 2, 2, 128, MH], f32, kind="ExternalInput")
    out_d = nc.dram_tensor("logits", [VM // 2, 128, 2 * TM], f16,
                           kind="ExternalOutput")
    PAIRS = [[2 * i, 2 * i + 1] for i in range(NCORES // 2)]

    with tile.TileContext(nc) as tc:
        with tc.tile_pool(name="per", bufs=1) as per, \
             tc.tile_pool(name="gpool", bufs=1) as gpool, \
             tc.tile_pool(name="lnst", bufs=2) as lnst:
            # persistent activation tiles
            xpre = [per.tile([128, T], f32, tag=f"xp{k}", name=f"xp{k}") for k in range(KH)]
            xt = [per.tile([128, T], f32, tag=f"xt{k}", name=f"xt{k}") for k in range(KH)]
            y1 = [per.tile([128, T], f32, tag=f"y1{k}", name=f"y1{k}") for k in range(KH)]
            hres = [per.tile([128, T], f32, tag=f"h{k}", name=f"h{k}") for k in range(KH)]
            yb1 = [per.tile([128, T], bf16, tag=f"yb1{k}", name=f"yb1{k}") for k in range(KH)]
            yb2 = [per.tile([128, T], bf16, tag=f"yb2{k}", name=f"yb2{k}") for k in range(KH)]
            g = [gpool.tile([128, T], bf16, tag=f"g{k}", name=f"g{k}") for k in range(MF)]
            half_f = per.tile([128, T], f32)
            nc.gpsimd.memset(half_f[:], 0.5)
            half = per.tile([128, T], bf16)
            nc.vector.tensor_copy(half[:], half_f[:])
            ones_f = per.tile([128, 1], f32)
            nc.gpsimd.memset(ones_f[:], 1.0)
            ones = per.tile([128, 1], bf16)
            nc.vector.tensor_copy(ones[:], ones_f[:])
            onesr_f = per.tile([1, 128], f32)
            nc.gpsimd.memset(onesr_f[:], 1.0)
            onesr = per.tile([1, 128], f32r)
            nc.vector.tensor_copy(onesr[:], onesr_f[:])
            mask = per.tile([128, 1], f32)
            nc.sync.dma_start(mask[:], mask_d.ap())
            m01f = per.tile([128, HALO], f32)
            nc.gpsimd.memset(m01f[:], 1.0)
            nc.vector.tensor_scalar_mul(m01f[:], m01f[:], mask[:, :1])
            mask01 = per.tile([128, HALO], bf16)
            nc.vector.tensor_copy(mask01[:], m01f[:])
            epsc = per.tile([128, 1], f32)
            nc.gpsimd.memset(epsc[:], EPS)
            if ln_scaled:
                lnt = per.tile([128, (2 * L + 2) * 2 * MH], f32)
                nc.sync.dma_start(
                    lnt[:],
                    lns_d.ap().rearrange("a b p m -> p (a b m)"))
            else:
                lnt = None

            def ln_cols(slot):
                if lnt is None:
                    return None, None
                off = slot * 2 * MH
                return lnt[:, off:off + MH], lnt[:, off + MH:off + 2 * MH]

            # ================= Embedding =================
            with tc.tile_pool(name="dramw", bufs=1, space="DRAM") as dramw:
                # tiny warm-up AllGather to absorb collective setup cost
                win = dramw.tile([128, 4], f32)
                nc.sync.dma_start(win[:], half_f[:, :4])
                wout = dramw.tile([2, 128, 4], f32)
                nc.gpsimd.collective_compute(
                    "AllGather", OP.bypass, replica_groups=PAIRS,
                    ins=[win.opt()], outs=[wout.opt()])
                for k in range(KH):
                    nc.sync.dma_start(xpre[k][:], xemb_d.ap()[k])

            # ================= Layers =================
            with tc.tile_pool(name="wret", bufs=6) as wret, \
                 tc.tile_pool(name="w1p", bufs=6) as w1p, \
                 tc.tile_pool(name="w2p", bufs=4) as w2p, \
                 tc.tile_pool(name="bias", bufs=2) as biasp, \
                 tc.tile_pool(name="tmp", bufs=4) as tmp, \
                 tc.tile_pool(name="sqp", bufs=3) as sqp, \
                 tc.tile_pool(name="psmm", bufs=4, space="PSUM") as psmm, \
                 tc.tile_pool(name="psst", bufs=2, space="PSUM") as ps_stat, \
                 tc.tile_pool(name="psbc", bufs=2, space="PSUM") as ps_bc:

                def stats_open():
                    # p_sy on partition 0, p_sq on partition 32: one PSUM bank
                    st = ps_stat.tile([33, T], f32, tag="pst", name="p_st")
                    return st

                def cast_sq(mt, src, yb):
                    """bf16 cast + square, both on the Scalar engine (Copy and
                    Square live in every activation table set)."""
                    nc.scalar.copy(yb[mt][:], src[mt][:])
                    sq = sqp.tile([128, T], bf16, tag="sq", name=f"sq{mt}")
                    nc.scalar.square(sq[:], yb[mt][:])
                    return sq

                def stats_mm(st_ps, mt, yb, sq, n):
                    nc.tensor.matmul(st_ps[0:1, :], ones[:], yb[mt][:],
                                     start=(mt == 0), stop=(mt == n - 1))
                    nc.tensor.matmul(st_ps[32:33, :], ones[:], sq[:],
                                     start=(mt == 0), stop=(mt == n - 1))

                def ln_finish(st_ps):
                    """Produce rb_sb ([128,T] rsqrt(var+eps)) and nmb_sb
                    ([128,T] negative mean) in SBUF."""
                    nm = lnst.tile([1, T], f32r, tag="nm", name="nm")
                    nc.vector.tensor_scalar_mul(nm[:], st_ps[0:1, :], -1.0 / H)
                    m2 = lnst.tile([1, T], f32, tag="m2", name="m2")
                    nc.vector.tensor_tensor(m2[:], nm[:].bitcast(f32),
                                            nm[:].bitcast(f32), OP.mult)
                    var = lnst.tile([1, T], f32r, tag="var", name="var")
                    nc.vector.scalar_tensor_tensor(var[:], st_ps[32:33, :],
                                                   1.0 / H, m2[:],
                                                   OP.mult, OP.subtract)
                    p_vb = ps_bc.tile([128, T], f32, tag="bc", name="p_vb")
                    nc.tensor.matmul(p_vb[:], onesr[:], var[:],
                                     start=True, stop=True)
                    p_nmb = ps_bc.tile([128, T], f32, tag="bc", name="p_nmb")
                    nc.tensor.matmul(p_nmb[:], onesr[:], nm[:],
                                     start=True, stop=True)
                    rb_sb = lnst.tile([128, T], f32, tag="rb", name="rb_sb")
                    nc.scalar.activation(rb_sb[:], p_vb[:],
                                         AF.Abs_reciprocal_sqrt, bias=epsc[:])
                    nmb_sb = lnst.tile([128, T], f32, tag="nmb", name="nmb_sb")
                    nc.scalar.copy(nmb_sb[:], p_nmb[:])
                    return {"rb": rb_sb, "nmb": nmb_sb}

                def apply_ln(mt, src, st, dst, slot):
                    """dst[mt] = (src[mt] + nmb)*rb (*s + b) on the Pool engine."""
                    z = tmp.tile([128, T], f32, tag="z", name="z")
                    nc.gpsimd.tensor_tensor(z[:], src[mt][:], st["nmb"][:], OP.add)
                    scol, bcol = ln_cols(slot)
                    if scol is None:
                        nc.gpsimd.tensor_tensor(dst[mt][:], z[:], st["rb"][:],
                                                OP.mult)
                    else:
                        z2 = tmp.tile([128, T], f32, tag="z2", name="z2")
                        nc.gpsimd.tensor_tensor(z2[:], z[:], st["rb"][:], OP.mult)
                        nc.gpsimd.tensor_scalar(dst[mt][:], z2[:],
                                                scol[:, mt:mt + 1],
                                                bcol[:, mt:mt + 1],
                                                OP.mult, OP.add)

                def epi(ps, st, out, func, bias):
                    """out = func(ps*rb + bias)."""
                    fin = tmp.tile([128, T], f32, tag="epf", name="epf")
                    nc.vector.tensor_tensor(fin[:], ps[:], st["rb"][:], OP.mult)
                    nc.scalar.activation(out, fin[:], func, bias=bias)

                # ---- embedding LN stats (emb acts as layer -1's LN2) ----
                emb_ps = stats_open()
                for k in range(KH):
                    sq = cast_sq(k, xpre, yb2)
                    stats_mm(emb_ps, k, yb2, sq, KH)
                st2 = ln_finish(emb_ps)
                # (st2_ps_pend, sq7) deferred from the previous layer's ffn2 so
                # the next ret head can be emitted between its PE instructions
                pend = None

                for l in range(L):
                    retb = biasp.tile([128, MH], f32, tag="retb")
                    nc.sync.dma_start(retb[:], retb_d.ap()[l])
                    b1 = biasp.tile([128, MF], f32, tag="b1")
                    nc.sync.dma_start(b1[:], b1_d.ap()[l])
                    b2 = biasp.tile([128, MH], f32, tag="b2")
                    nc.sync.dma_start(b2[:], b2_d.ap()[l])

                    # ---------- retention ----------
                    st1_ps = stats_open()
                    sq_pend = {}      # mt -> sq tile (stats emitted at lag 2)
                    for mt in range(MH):
                        wt = wret.tile([128, KH * 128], bf16, tag="wret")
                        nc.sync.dma_start(wt[:], retw_d.ap()[l, mt])
                        ps = psmm.tile([128, T], f32, tag="mm")
                        if mt == 0 and pend is not None:
                            # head: kc0-6 fill the PE while yb2[7]'s chain and
                            # the previous LN2 finish complete
                            for kc in range(7):
                                nc.tensor.matmul(
                                    ps[:], wt[:, kc * 128:(kc + 1) * 128],
                                    yb2[kc][:], start=(kc == 0), stop=False)
                            p_st2_ps, p_sq7 = pend
                            pend = None
                            stats_mm(p_st2_ps, MH - 1, yb2, p_sq7, MH)
                            st2 = ln_finish(p_st2_ps)
                            nc.tensor.matmul(
                                ps[:], wt[:, 7 * 128:8 * 128], yb2[7][:],
                                start=False, stop=True)
                        else:
                            for kc in range(KH):
                                nc.tensor.matmul(
                                    ps[:], wt[:, kc * 128:(kc + 1) * 128],
                                    yb2[kc][:], start=(kc == 0),
                                    stop=(kc == KH - 1))
                        s = tmp.tile([128, T], bf16, tag="sig", name="sig")
                        epi(ps[:], st2, s[:], AF.Sigmoid, retb[:, mt:mt + 1])
                        nc.gpsimd.tensor_tensor(
                            s[:, :HALO], s[:, :HALO], mask01[:], OP.mult)
                        # xt[mt] = LN2(xpre[mt]) just in time for the residual
                        apply_ln(mt, xpre, st2, xt,
                                 (2 * l) if ln_scaled else 0)
                        stt = tmp.tile([128, T], bf16, tag="scan", name="scan")
                        nc.vector.tensor_tensor_scan(
                            stt[:], half[:], s[:], 0.0, OP.mult, OP.add)
                        nc.vector.scalar_tensor_tensor(
                            y1[mt][:], stt[:], 0.5, xt[mt][:], OP.mult, OP.add)
                        sq_pend[mt] = cast_sq(mt, y1, yb1)

                    # ffn1 head: 4 m-tiles kc-major, interleaved with the LN1
                    # stats so the PE consumes the retention chain's outputs
                    # in production order instead of stalling on them.  Dummy
                    # matmuls (always-ready inputs, dead PSUM target) pad the
                    # PE stream so it never idles waiting for the chain —
                    # idling drops the PE to a lower p-state and the first
                    # ~3us after each gap would run at half clock.
                    NG = 4
                    fwt = []
                    fps = []
                    for mt in range(NG):
                        wt = w1p.tile([128, KH * 128], bf16, tag="w1")
                        nc.sync.dma_start(wt[:], w1_d.ap()[l, mt])
                        fwt.append(wt)
                        ps = psmm.tile([128, T], f32, tag="mm")
                        fps.append(ps)
                    for kc in range(KH):
                        for mt in range(NG):
                            nc.tensor.matmul(
                                fps[mt][:], fwt[mt][:, kc * 128:(kc + 1) * 128],
                                yb1[kc][:], start=(kc == 0), stop=(kc == KH - 1))
                        stats_mm(st1_ps, kc, yb1, sq_pend.pop(kc), MH)
                    st1 = ln_finish(st1_ps)

                    # ---------- FFN1 ----------
                    for mt in range(MF):
                        if mt < NG:
                            ps = fps[mt]
                        else:
                            wt = w1p.tile([128, KH * 128], bf16, tag="w1")
                            nc.sync.dma_start(wt[:], w1_d.ap()[l, mt])
                            ps = psmm.tile([128, T], f32, tag="mm")
                            for kc in range(KH):
                                nc.tensor.matmul(
                                    ps[:], wt[:, kc * 128:(kc + 1) * 128], yb1[kc][:],
                                    start=(kc == 0), stop=(kc == KH - 1))
                        epi(ps[:], st1, g[mt][:], AF.Gelu_apprx_tanh,
                            b1[:, mt:mt + 1])
                        if mt < MH:
                            # h[mt] = LN1(y1[mt]) for the ffn2 residual
                            apply_ln(mt, y1, st1, hres,
                                     (2 * l + 1) if ln_scaled else 0)

                    # ---------- FFN2 ----------
                    # last layer (identity path): only the bf16 casts are
                    # needed — the final LN scale is recomputed from the
                    # gathered casts on the LM side, so skip stats entirely
                    last_id = (l == L - 1) and not ln_scaled
                    st2_ps = None if last_id else stats_open()
                    sq_pend = {}
                    for mt in range(MH):
                        wt = w2p.tile([128, MF * 128], bf16, tag="w2")
                        nc.sync.dma_start(wt[:], w2_d.ap()[l, mt])
                        ps = psmm.tile([128, T], f32, tag="mm")
                        for kc in range(MF):
                            nc.tensor.matmul(
                                ps[:], wt[:, kc * 128:(kc + 1) * 128], g[kc][:],
                                start=(kc == 0), stop=(kc == MF - 1))
                        # xpre' = (ffn + b2) + h
                        nc.vector.scalar_tensor_tensor(
                            xpre[mt][:], ps[:], b2[:, mt:mt + 1],
                            hres[mt][:], OP.add, OP.add)
                        if last_id:
                            nc.scalar.copy(yb2[mt][:], xpre[mt][:])
                            continue
                        sq_pend[mt] = cast_sq(mt, xpre, yb2)
                        if mt >= 2:
                            stats_mm(st2_ps, mt - 2, yb2, sq_pend.pop(mt - 2), MH)

                    # stats(6) now; stats(7) + ln2 finish are deferred into the
                    # next layer's ret head so the PE keeps streaming
                    if not last_id:
                        stats_mm(st2_ps, MH - 2, yb2, sq_pend.pop(MH - 2), MH)
                        if l < L - 1:
                            pend = (st2_ps, sq_pend.pop(MH - 1))
                        else:
                            stats_mm(st2_ps, MH - 1, yb2, sq_pend.pop(MH - 1), MH)
                            st2 = ln_finish(st2_ps)

                # ---- final LN (identity path: LM reads xpre*rb directly) ----
                if ln_scaled:
                    # materialize the scaled LN2 output, then run the final LN
                    for k in range(KH):
                        apply_ln(k, xpre, st2, xt, 2 * L)
                    stf_ps = stats_open()
                    sq_pend = {}
                    for k in range(KH):
                        sq = cast_sq(k, xt, yb1)
                        stats_mm(stf_ps, k, yb1, sq, KH)
                    stf = ln_finish(stf_ps)
                    for k in range(KH):
                        apply_ln(k, xt, stf, hres, 2 * L + 1)
                    xlm = yb2
                    for k in range(KH):
                        nc.scalar.copy(xlm[k][:], hres[k][:])
                else:
                    # raw bf16 casts are gathered; the final LN rsqrt is
                    # recomputed on the receiving side for all pair tokens
                    xlm = yb2

            # ===== LM head: 2-way vocab shard x pair token gather =====
            with tc.tile_pool(name="dram", bufs=1, space="DRAM") as dramp, \
                 tc.tile_pool(name="lmx", bufs=1) as lmx, \
                 tc.tile_pool(name="lmsq", bufs=2) as lmsq, \
                 tc.tile_pool(name="lmw", bufs=10) as lmwp, \
                 tc.tile_pool(name="lmo", bufs=4) as lmo, \
                 tc.tile_pool(name="pslm", bufs=6, space="PSUM") as pslm, \
                 tc.tile_pool(name="pslst", bufs=1, space="PSUM") as ps_lst:
                bnc = dramp.tile([H, TM], bf16)
                for k in range(KH):
                    nc.sync.dma_start(bnc[k * 128:(k + 1) * 128, :],
                                      xlm[k][:, HALO:T])
                xg = dramp.tile([2, H, TM], bf16)
                nc.gpsimd.collective_compute(
                    "AllGather", OP.bypass, replica_groups=PAIRS,
                    ins=[bnc.opt()], outs=[xg.opt()])
                rhs = []
                for k in range(KH):
                    t_ = lmx.tile([128, 2 * TM], bf16, tag=f"rhs{k}",
                                  name=f"rhs{k}")
                    for r in range(2):
                        nc.sync.dma_start(
                            t_[:, r * TM:(r + 1) * TM],
                            xg[r, k * 128:(k + 1) * 128, :])
                    rhs.append(t_)
                if not ln_scaled:
                    # recompute the final-LN rsqrt for the pair's 512 tokens
                    lm_st = ps_lst.tile([33, 2 * TM], f32, tag="lmst",
                                        name="lm_st")
                    for k in range(KH):
                        sq = lmsq.tile([128, 2 * TM], bf16, tag="lsq",
                                       name="lsq")
                        nc.scalar.square(sq[:], rhs[k][:])
                        nc.tensor.matmul(lm_st[0:1, :], ones[:], rhs[k][:],
                                         start=(k == 0), stop=(k == KH - 1))
                        nc.tensor.matmul(lm_st[32:33, :], ones[:], sq[:],
                                         start=(k == 0), stop=(k == KH - 1))
                    nm = lmsq.tile([1, 2 * TM], f32r, tag="lnm", name="lnm")
                    nc.vector.tensor_scalar_mul(nm[:], lm_st[0:1, :], -1.0 / H)
                    m2 = lmsq.tile([1, 2 * TM], f32, tag="lm2", name="lm2")
                    nc.vector.tensor_tensor(m2[:], nm[:].bitcast(f32),
                                            nm[:].bitcast(f32), OP.mult)
                    var = lmsq.tile([1, 2 * TM], f32r, tag="lvar", name="lvar")
                    nc.vector.scalar_tensor_tensor(var[:], lm_st[32:33, :],
                                                   1.0 / H, m2[:],
                                                   OP.mult, OP.subtract)
                    p_vb = ps_lst.tile([128, 2 * TM], f32, tag="lbc",
                                       name="p_vb")
                    nc.tensor.matmul(p_vb[:], onesr[:], var[:],
                                     start=True, stop=True)
                    rb = lmsq.tile([128, 2 * TM], f32, tag="lrb", name="lrb")
                    nc.scalar.activation(rb[:], p_vb[:],
                                         AF.Abs_reciprocal_sqrt, bias=epsc[:])
                    for k in range(KH):
                        nrm = lmx.tile([128, 2 * TM], bf16, tag=f"nrm{k}",
                                       name=f"nrm{k}")
                        nc.vector.tensor_tensor(nrm[:], rhs[k][:], rb[:],
                                                OP.mult)
                        rhs[k] = nrm
                for mt in range(VM // 2):
                    wt = lmwp.tile([128, KH * 128], bf16, tag="lmw")
                    nc.sync.dma_start(wt[:], lmw_d.ap()[mt])
                    ps = pslm.tile([128, 2 * TM], f32, tag="lm")
                    for kc in range(KH):
                        nc.tensor.matmul(
                            ps[:], wt[:, kc * 128:(kc + 1) * 128], rhs[kc][:],
                            start=(kc == 0), stop=(kc == KH - 1))
                    ob = lmo.tile([128, 2 * TM], f16, tag="ob")
                    nc.any.tensor_copy(ob[:], ps[:])
                    nc.sync.dma_start(out_d.ap()[mt], ob[:])

    nc.compile()
    return nc


def _prep_inputs(inputs, ln_scaled):
    import ml_dtypes
    ids = np.asarray(inputs["input_ids"], np.int32)          # [B, S]
    retw_raw = [np.asarray(inputs["ret_W"][l], np.float32) for l in range(L)]
    w1_raw = [np.asarray(inputs["ffn_W1"][l], np.float32) for l in range(L)]
    retb_raw = [np.asarray(inputs["ret_b"][l], np.float32) for l in range(L)]
    b1_raw = [np.asarray(inputs["ffn_b1"][l], np.float32) for l in range(L)]
    if ln_scaled:
        # fold LN scale/bias of the LN feeding each fused GEMM into W / bias
        for l in range(L):
            s_in = (np.asarray(inputs["emb_ln_s"], np.float32) if l == 0
                    else np.asarray(inputs["ln2_s"][l - 1], np.float32))
            b_in = (np.asarray(inputs["emb_ln_b"], np.float32) if l == 0
                    else np.asarray(inputs["ln2_b"][l - 1], np.float32))
            retb_raw[l] = retb_raw[l] + b_in @ retw_raw[l]
            retw_raw[l] = retw_raw[l] * s_in[:, None]
            s1 = np.asarray(inputs["ln1_s"][l], np.float32)
            b1_ = np.asarray(inputs["ln1_b"][l], np.float32)
            b1_raw[l] = b1_raw[l] + b1_ @ w1_raw[l]
            w1_raw[l] = w1_raw[l] * s1[:, None]
    # column-mean-center: W~ = W - 1 (x) colmean(W) makes the GEMM
    # invariant to the per-token mean of its input
    retw_raw = [w - w.mean(0, keepdims=True) for w in retw_raw]
    w1_raw = [w - w.mean(0, keepdims=True) for w in w1_raw]
    retw = np.stack([_swz(w) for w in retw_raw]).astype(ml_dtypes.bfloat16)
    w1 = np.stack([_swz(w) for w in w1_raw]).astype(ml_dtypes.bfloat16)
    w2 = np.stack([_swz(np.asarray(inputs["ffn_W2"][l], np.float32))
                   for l in range(L)]).astype(ml_dtypes.bfloat16)
    retb = np.stack([_cols(v, MH) for v in retb_raw])
    b1 = np.stack([_cols(v, MF) for v in b1_raw])
    b2 = np.stack([_cols(np.asarray(inputs["ffn_b2"][l], np.float32), MH)
                   for l in range(L)])
    lmw_full = np.asarray(inputs["lm_W"], np.float32)         # [H, V]
    if not ln_scaled:
        lmw_full = lmw_full - lmw_full.mean(0, keepdims=True)
    pos_emb = np.asarray(inputs["pos_emb"], np.float32)       # [S, H]
    wemb = np.asarray(inputs["word_emb"], np.float32)

    common = {
        "retw": retw, "retb": retb,
        "w1": w1, "b1": b1, "w2": w2, "b2": b2,
    }
    lmw_halves = [
        _swz(np.ascontiguousarray(lmw_full[:, h * (V // 2):(h + 1) * (V // 2)])
             ).astype(ml_dtypes.bfloat16)
        for h in range(2)
    ]
    if ln_scaled:
        slots = [( np.asarray(inputs["emb_ln_s"], np.float32),
                   np.asarray(inputs["emb_ln_b"], np.float32))]
        for l in range(L):
            slots.append((np.asarray(inputs["ln1_s"][l], np.float32),
                          np.asarray(inputs["ln1_b"][l], np.float32)))
            slots.append((np.asarray(inputs["ln2_s"][l], np.float32),
                          np.asarray(inputs["ln2_b"][l], np.float32)))
        slots.append((np.asarray(inputs["fin_ln_s"], np.float32),
                      np.asarray(inputs["fin_ln_b"], np.float32)))
        lns = np.stack([np.stack([_cols(s, MH), _cols(b, MH)]) for s, b in slots])
        common["lns"] = lns

    in_maps = []
    for c in range(NCORES):
        b = c // (NCORES // B)
        s0 = TM * (c % (NCORES // B))
        if s0 == 0:
            hids = ids[b, 0:HALO]
            hpos = np.arange(HALO)
        else:
            hids = ids[b, s0 - HALO:s0]
            hpos = np.arange(s0 - HALO, s0)
        cids = np.concatenate([hids, ids[b, s0:s0 + TM]]).astype(np.int64)
        cpos = np.concatenate([hpos, np.arange(s0, s0 + TM)])
        xemb = (wemb[cids] + pos_emb[cpos]).astype(np.float32)   # [T, H]
        m = dict(common)
        m["mask"] = np.full((128, 1), 0.0 if s0 == 0 else 1.0, np.float32)
        m["xemb"] = np.ascontiguousarray(xemb.T.reshape(KH, 128, T))
        m["lmw"] = lmw_halves[c % 2]
        in_maps.append(m)
    return in_maps


def kernel(**inputs):
    trivial = all(
        np.allclose(np.asarray(inputs[k]), 1.0) for k in
        ("emb_ln_s", "ln1_s", "ln2_s", "fin_ln_s")
    ) and all(
        np.allclose(np.asarray(inputs[k]), 0.0) for k in
        ("emb_ln_b", "ln1_b", "ln2_b", "fin_ln_b")
    )
    ln_scaled = not trivial

    if ln_scaled not in _compiled:
        _compiled[ln_scaled] = _build(ln_scaled)
    nc = _compiled[ln_scaled]

    in_maps = _prep_inputs(inputs, ln_scaled)
    trace = bool(_os.environ.get("KERNEL_TRACE"))
    if trace:
        _maybe_install_trace_hook()
    res = bass_utils.run_bass_kernel_spmd(
        nc, in_maps, core_ids=list(range(NCORES)), trace=trace)
    global LAST_EXEC_NS
    LAST_EXEC_NS = res.exec_time_ns

    logits = np.empty((B, S, V), np.float32)
    for c in range(NCORES):
        b = c // (NCORES // B)
        s0 = 2 * TM * ((c // 2) % 2)
        vh = c % 2
        lg = np.asarray(res.results[c]["logits"])  # [125, 128, 512] f16
        logits[b, s0:s0 + 2 * TM, vh * (V // 2):(vh + 1) * (V // 2)] = \
            lg.reshape(V // 2, 2 * TM).astype(np.float32).T
    return logits
